# revision 11
# baseline (speedup 1.0000x reference)
"""Trainium2 Bass kernel for single-head DotProductAttention with softmax over
the *query* axis (axis=-2) and causal mask, returning (out, attention).

Reference semantics (B=4, S=2048, D_MODEL=1024, D_K=D_V=128):
    Q = x @ w_q; K = x @ w_k; V = x @ w_v
    scores[b,q,k] = (Q[b,q] . K[b,k]) / sqrt(128),  masked to -inf for k > q
    attention = softmax(scores, axis=-2)            # normalize over q per column k
    out[b,q,v] = sum_k attention[b,q,k] V[b,k,v]

Sharding: 8 cores = (batch b, half h). Core (b,h) owns 8 interleaved k-chunks
of 128 columns at global k0 = 256*j + 128*h (j=0..7), which balances the
causal triangle across the pair of cores sharing a batch.

Device layout: scores are computed transposed, S^T[k_part, q_free], so the
softmax over q is a free-axis reduction (per-partition max / fused exp+accum
on the scalar engine). Causality: chunk j only computes q >= 256*j. The
128x256 diagonal-block mask is a data input so the program is SPMD-uniform.

Matmul precision/speed: the PE runs fp32 matmuls as 2 half-rate passes
(4 cyc/row). Instead, Q/K/V/scores use a *split fp16* scheme at 3 cyc/row
with better accuracy: x = hi + 2^-11*lo with hi = fp16(x) and
lo = fp16((x-hi)*2^11) (scaling keeps lo in fp16 normal range). Then
x@w = hi@wh + 2^-11*(hi@wl' + lo'@wh) + O(2^-22); the 2^-11 recombination
is fused into the PSUM-evacuation op. The residual (~2^-22 per product) is
below the error of the PE's own fp32 mode. out's A~@V matmul stays fp32.

Host does layout-only work (transpose/fp16 split/gather/scatter/scale); all
matmul FLOPs stay on device.
"""

import sys
from contextlib import ExitStack

import numpy as np

if "/opt/trn_rl_repo" not in sys.path:
    sys.path.insert(0, "/opt/trn_rl_repo")

import concourse.bass as bass
import concourse.tile as tile
from concourse import bacc, mybir
from concourse.bass_utils import run_bass_kernel_spmd
from concourse.masks import make_identity

B, S, DM, DK, DV = 4, 2048, 1024, 128, 128
NCHUNK = 8          # k-chunks per core, 128 wide
CW = 128            # chunk width
QSTEP = 256         # q start of chunk j is QSTEP*j
INV_SQRT_DK = 1.0 / np.sqrt(np.float32(DK))
NEG = -1.0e30
LSH = 2048.0        # 2^11 lo-part scale
LSHI = float(1.0 / 2048.0)

_CACHE = {}


def _build_program():
    f32 = mybir.dt.float32
    f16 = mybir.dt.float16
    nc = bacc.Bacc("TRN2", target_bir_lowering=False, debug=False, num_devices=8)

    def din(name, shape, dt=f16):
        return nc.dram_tensor(name, shape, dt, kind="ExternalInput").ap()

    xth = din("xth", [DM, S])
    xtl = din("xtl", [DM, S])
    xkh = din("xkh", [DM, NCHUNK * CW])
    xkl = din("xkl", [DM, NCHUNK * CW])
    wqh, wql = din("wqh", [DM, DK]), din("wql", [DM, DK])
    wkh, wkl = din("wkh", [DM, DK]), din("wkl", [DM, DK])
    wvh, wvl = din("wvh", [DM, DV]), din("wvl", [DM, DV])
    dmask = din("dmask", [CW, QSTEP], f32)
    # A~^T chunk-major: rows j*128..(j+1)*128 = chunk j (k within chunk),
    # cols = q. Only q >= 256*j is written; the rest stays zero.
    attn = nc.dram_tensor("attn", [NCHUNK * CW, S], f32, kind="ExternalOutput").ap()
    ot = nc.dram_tensor("ot", [DV, S], f32, kind="ExternalOutput").ap()
    recip_out = nc.dram_tensor("recip", [CW, NCHUNK], f32, kind="ExternalOutput").ap()

    MC = DM // 128  # 8 contraction chunks over d_model

    with tile.TileContext(nc) as tc, ExitStack() as ctx:
        pers = ctx.enter_context(tc.tile_pool(name="pers", bufs=1))
        QT = pers.tile([128, S], f32, name="QT", tag="QT")
        QTh = pers.tile([128, S], f16, name="QTh", tag="QTh")
        QTl = pers.tile([128, S], f16, name="QTl", tag="QTl")
        KTh = pers.tile([128, NCHUNK * CW], f16, name="KTh", tag="KTh")
        KTl = pers.tile([128, NCHUNK * CW], f16, name="KTl", tag="KTl")
        Vt = [pers.tile([128, DV], f32, name=f"V{j}", tag=f"V{j}")
              for j in range(NCHUNK)]
        AT = [pers.tile([128, S - QSTEP * j], f32, name=f"AT{j}", tag=f"AT{j}")
              for j in range(NCHUNK)]
        RECIP = pers.tile([128, NCHUNK], f32, name="RECIP", tag="RECIP")
        DEN = pers.tile([128, NCHUNK], f32, name="DEN", tag="DEN")
        dm = pers.tile([CW, QSTEP], f32, name="dmask", tag="dmask")
        ident = pers.tile([128, 128], f32, name="ident", tag="ident")

        # HAM warmup: ~40 back-to-back tiny matmuls get the PE clock to
        # 2.4GHz (~4us of sustained busy) while input DMAs stream in.
        with tc.tile_pool(name="psW", bufs=1, space="PSUM") as psW:
            make_identity(nc, ident[:])
            wps = psW.tile([128, 128], f32, name="warm", tag="warm")
            for r in range(40):
                nc.tensor.matmul(wps[:], ident[:], ident[:],
                                 start=True, stop=True)

        # ---------------- phase B: projections (split-fp16) ----------------
        with tc.tile_pool(name="pB", bufs=1) as pB, \
             tc.tile_pool(name="psB", bufs=2, space="PSUM") as psB:
            # DMA issue order = consumption order.
            def load_w(w, nm):
                out = []
                for m in range(MC):
                    t = pB.tile([128, 128], f16, name=f"{nm}{m}", tag=f"{nm}{m}")
                    nc.sync.dma_start(t[:], w[m * 128:(m + 1) * 128, :])
                    out.append(t)
                return out

            WQh = load_w(wqh, "wqh")
            WQl = load_w(wql, "wql")
            XTH, XTL = {}, {}
            for s in range(4):
                for nm, src, dst in (("xth", xth, XTH), ("xtl", xtl, XTL)):
                    for m in range(MC):
                        t = pB.tile([128, 512], f16, name=f"{nm}{m}_{s}",
                                    tag=f"{nm}{m}_{s}")
                        nc.sync.dma_start(t[:], src[m * 128:(m + 1) * 128,
                                                    s * 512:(s + 1) * 512])
                        dst[(m, s)] = t
            WKh = load_w(wkh, "wkh")
            WKl = load_w(wkl, "wkl")
            WVh = load_w(wvh, "wvh")
            WVl = load_w(wvl, "wvl")
            XKH, XKL = {}, {}
            for s in range(2):
                for nm, src, dst in (("xkh", xkh, XKH), ("xkl", xkl, XKL)):
                    for m in range(MC):
                        t = pB.tile([128, 512], f16, name=f"{nm}{m}_{s}",
                                    tag=f"{nm}{m}_{s}")
                        nc.sync.dma_start(t[:], src[m * 128:(m + 1) * 128,
                                                    s * 512:(s + 1) * 512])
                        dst[(m, s)] = t
            nc.sync.dma_start(dm[:], dmask[:, :])

            def proj(dst_ap, seg, WH, WL, XH, XL, s):
                """dst_ap[:, :] = (WH,WL).T @ (XH + 2^-11 XL) for segment s."""
                psm = psB.tile([128, 512], f32, name="psm", tag="psm")
                psl = psB.tile([128, 512], f32, name="psl", tag="psl")
                for m in range(MC):
                    nc.tensor.matmul(psm[:], WH[m][:], XH[(m, s)][:],
                                     start=(m == 0), stop=(m == MC - 1))
                for m in range(MC):
                    nc.tensor.matmul(psl[:], WL[m][:], XH[(m, s)][:],
                                     start=(m == 0), stop=False,
                                     skip_group_check=True)
                for m in range(MC):
                    nc.tensor.matmul(psl[:], WH[m][:], XL[(m, s)][:],
                                     start=False, stop=(m == MC - 1),
                                     skip_group_check=True)
                # combine: dst = psm + 2^-11 * psl (DVE may read only one
                # PSUM operand per op, so evacuate psm first via ACT)
                nc.scalar.copy(dst_ap, psm[:])
                nc.vector.scalar_tensor_tensor(
                    dst_ap, psl[:], LSHI, dst_ap,
                    op0=mybir.AluOpType.mult, op1=mybir.AluOpType.add)

            # Q/K values are O(32): their fp16 lo-parts (~32*2^-11) are well
            # inside fp16 normal range, so no scaling needed on-device.
            for s in range(4):
                proj(QT[:, s * 512:(s + 1) * 512], s, WQh, WQl, XTH, XTL, s)
            for s in range(4):
                sl = slice(s * 512, (s + 1) * 512)
                nc.scalar.copy(QTh[:, sl], QT[:, sl])
                nc.vector.scalar_tensor_tensor(
                    QTl[:, sl], QTh[:, sl], -1.0, QT[:, sl],
                    op0=mybir.AluOpType.mult, op1=mybir.AluOpType.add)

            KT = pers.tile([128, NCHUNK * CW], f32, name="KT", tag="KT")
            for s in range(2):
                proj(KT[:, s * 512:(s + 1) * 512], s, WKh, WKl, XKH, XKL, s)
            for s in range(2):
                sl = slice(s * 512, (s + 1) * 512)
                nc.scalar.copy(KTh[:, sl], KT[:, sl])
                nc.vector.scalar_tensor_tensor(
                    KTl[:, sl], KTh[:, sl], -1.0, KT[:, sl],
                    op0=mybir.AluOpType.mult, op1=mybir.AluOpType.add)

            # V^T then transpose per chunk to V[kl, v]
            VTT = pers.tile([128, NCHUNK * CW], f32, name="VTT", tag="VTT")
            for s in range(2):
                proj(VTT[:, s * 512:(s + 1) * 512], s, WVh, WVl, XKH, XKL, s)
            for j in range(NCHUNK):
                ps = psB.tile([128, 512], f32, name="psm", tag="psm")
                nc.tensor.transpose(ps[:, 0:128],
                                    VTT[:, j * CW:(j + 1) * CW], ident[:])
                nc.vector.tensor_copy(Vt[j][:], ps[:, 0:128])

        # ---------------- phases C (scores/softmax) + D (out) ----------------
        with tc.tile_pool(name="pC", bufs=2) as pC, \
             tc.tile_pool(name="psS", bufs=2, space="PSUM") as psS, \
             tc.tile_pool(name="psO", bufs=1, space="PSUM") as psO:

            pso = psO.tile([128, S], f32, name="O", tag="O")
            OT = pers.tile([128, S], f32, name="OT", tag="OT")

            for j in range(NCHUNK):
                qs0 = QSTEP * j
                lhsh = KTh[:, j * CW:(j + 1) * CW]
                lhsl = KTl[:, j * CW:(j + 1) * CW]
                maxs = pC.tile([128, 8], f32, name="maxs", tag="maxs")
                nseg = 0
                for qs in range(qs0, S, 512):
                    w = min(512, S - qs)
                    # all three fp16 passes accumulate into one PSUM group
                    # (the Q/K lo-parts are unscaled)
                    psm = psS.tile([128, 512], f32, name="Sm", tag="Sm")
                    nc.tensor.matmul(psm[:, 0:w], lhsh, QTh[:, qs:qs + w],
                                     start=True, stop=False)
                    nc.tensor.matmul(psm[:, 0:w], lhsl, QTh[:, qs:qs + w],
                                     start=False, stop=False)
                    nc.tensor.matmul(psm[:, 0:w], lhsh, QTl[:, qs:qs + w],
                                     start=False, stop=True)
                    dst = AT[j][:, qs - qs0:qs - qs0 + w]
                    nc.scalar.copy(dst, psm[:, 0:w])
                    if qs == qs0:
                        nc.vector.tensor_add(AT[j][:, 0:QSTEP],
                                             AT[j][:, 0:QSTEP], dm[:])
                    nc.vector.reduce_max(maxs[:, nseg:nseg + 1],
                                         AT[j][:, qs - qs0:qs - qs0 + w],
                                         axis=mybir.AxisListType.X)
                    nseg += 1
                m = pC.tile([128, 1], f32, name="m", tag="m")
                nc.vector.reduce_max(m[:], maxs[:, 0:nseg],
                                     axis=mybir.AxisListType.X)
                bias = pC.tile([128, 1], f32, name="bias", tag="bias")
                nc.scalar.mul(bias[:], m[:], -INV_SQRT_DK)
                nc.scalar.activation(AT[j][:], AT[j][:],
                                     mybir.ActivationFunctionType.Exp,
                                     bias=bias[:], scale=float(INV_SQRT_DK),
                                     accum_out=DEN[:, j:j + 1])
                nc.vector.reciprocal(RECIP[:, j:j + 1], DEN[:, j:j + 1])

                # store A~^T chunk rows directly (host transposes)
                nc.sync.dma_start(attn[j * CW:(j + 1) * CW, qs0:S], AT[j][:])

                # inline O^T accumulation for this chunk (fp32)
                vs = pers.tile([128, DV], f32, name=f"VS{j}", tag=f"VS{j}")
                nc.vector.tensor_scalar_mul(vs[:], Vt[j][:], RECIP[:, j:j + 1])
                pieces = []
                qs = qs0
                if qs % 512:
                    pieces.append((qs, 512 - qs % 512))
                    qs += 512 - qs % 512
                while qs < S:
                    pieces.append((qs, min(512, S - qs)))
                    qs += 512
                for qs, w in pieces:
                    nc.tensor.matmul(pso[:, qs:qs + w], vs[:],
                                     AT[j][:, qs - qs0:qs - qs0 + w],
                                     start=(j == 0), stop=(j == NCHUNK - 1),
                                     skip_group_check=True)
                # O^T 512-seg t is complete once chunk 2t+1 contributed:
                # evacuate + store early so the tail only carries the last seg.
                if j % 2 == 1:
                    t = j // 2
                    sl = slice(t * 512, (t + 1) * 512)
                    nc.scalar.copy(OT[:, sl], pso[:, sl])
                    nc.sync.dma_start(ot[:, sl], OT[:, sl])

            nc.sync.dma_start(recip_out[:, :], RECIP[:])

    nc.compile()
    return nc


def _get_program():
    if "nc" not in _CACHE:
        _CACHE["nc"] = _build_program()
    return _CACHE["nc"]


def _split16(a):
    hi = a.astype(np.float16)
    lo = ((a - hi.astype(np.float32)) * np.float32(LSH)).astype(np.float16)
    return hi, lo


def _core_inputs(x_q, splits, c):
    b, h = divmod(c, 2)
    xth, xtl = splits[f"xt{b}"]
    cols = np.concatenate(
        [np.arange(QSTEP * j + CW * h, QSTEP * j + CW * h + CW)
         for j in range(NCHUNK)])
    p = np.arange(CW)
    ql = np.arange(QSTEP)
    dmask = np.where(ql[None, :] >= (CW * h + p)[:, None],
                     np.float32(0.0), np.float32(NEG)).astype(np.float32)
    d = {"xth": xth, "xtl": xtl,
         "xkh": np.ascontiguousarray(xth[:, cols]),
         "xkl": np.ascontiguousarray(xtl[:, cols]),
         "dmask": dmask}
    for nm in ("wq", "wk", "wv"):
        d[nm + "h"], d[nm + "l"] = splits[nm]
    return d


def kernel(x_q, w_q, w_k, w_v, _trace=False, _results_hook=None):
    x_q = np.ascontiguousarray(x_q, dtype=np.float32)

    splits = {}
    for b in range(B):
        splits[f"xt{b}"] = _split16(np.ascontiguousarray(x_q[b].T))
    splits["wq"] = _split16(np.asarray(w_q, dtype=np.float32))
    splits["wk"] = _split16(np.asarray(w_k, dtype=np.float32))
    splits["wv"] = _split16(np.asarray(w_v, dtype=np.float32))

    nc = _get_program()
    in_maps = [_core_inputs(x_q, splits, c) for c in range(8)]
    res = run_bass_kernel_spmd(nc, in_maps, list(range(8)), trace=_trace)
    if _results_hook is not None:
        _results_hook(res)

    attention = np.zeros((B, S, S), dtype=np.float32)
    out = np.empty((B, S, DV), dtype=np.float32)
    for b in range(B):
        o = None
        for h in range(2):
            c = 2 * b + h
            r = res.results[c]
            a_raw = r["attn"]                 # [1024, S] chunk-major, [k, q]
            rc = r["recip"]                   # [128, 8]
            for j in range(NCHUNK):
                k0 = QSTEP * j + CW * h
                qs0 = QSTEP * j
                blk = a_raw[j * CW:(j + 1) * CW, qs0:] * rc[:, j][:, None]
                attention[b][qs0:, k0:k0 + CW] = blk.T
            oc = r["ot"]                      # [DV, S]
            o = oc if o is None else o + oc
        out[b] = o.T
    return out, attention


# revision 16
# speedup vs baseline: 1.2384x; 1.2384x over previous
"""Trainium2 Bass kernel for single-head DotProductAttention with softmax over
the *query* axis (axis=-2) and causal mask, returning (out, attention).

Reference semantics (B=4, S=2048, D_MODEL=1024, D_K=D_V=128):
    Q = x @ w_q; K = x @ w_k; V = x @ w_v
    scores[b,q,k] = (Q[b,q] . K[b,k]) / sqrt(128),  masked to -inf for k > q
    attention = softmax(scores, axis=-2)            # normalize over q per column k
    out[b,q,v] = sum_k attention[b,q,k] V[b,k,v]

Sharding: 8 cores = (batch b, half h). Core (b,h) owns 8 interleaved k-chunks
of 128 columns at global k0 = 256*j + 128*h (j=0..7), which balances the
causal triangle across the pair of cores sharing a batch.

Device layout: scores are computed transposed, S^T[k_part, q_free], so the
softmax over q is a free-axis reduction (per-partition max / fused exp+accum
on the scalar engine). Causality: chunk j only computes q >= 256*j. The
128x256 diagonal-block mask is a data input so the program is SPMD-uniform.

Matmul precision/speed: the PE runs fp32 matmuls as 2 half-rate passes
(4 cyc/row). Instead, Q/K/V/scores use a *split fp16* scheme at 3 cyc/row
with better accuracy: x = hi + 2^-11*lo with hi = fp16(x) and
lo = fp16((x-hi)*2^11) (scaling keeps lo in fp16 normal range). Then
x@w = hi@wh + 2^-11*(hi@wl' + lo'@wh) + O(2^-22); the 2^-11 recombination
is fused into the PSUM-evacuation op. The residual (~2^-22 per product) is
below the error of the PE's own fp32 mode. out's A~@V matmul stays fp32.

Host does layout-only work (transpose/fp16 split/gather/scatter/scale); all
matmul FLOPs stay on device.
"""

import sys
from contextlib import ExitStack

import numpy as np

if "/opt/trn_rl_repo" not in sys.path:
    sys.path.insert(0, "/opt/trn_rl_repo")

import concourse.bass as bass
import concourse.tile as tile
from concourse import bacc, mybir
from concourse.bass_utils import run_bass_kernel_spmd
from concourse.masks import make_identity

B, S, DM, DK, DV = 4, 2048, 1024, 128, 128
NCHUNK = 8          # k-chunks per core, 128 wide
CW = 128            # chunk width
QSTEP = 256         # q start of chunk j is QSTEP*j
INV_SQRT_DK = 1.0 / np.sqrt(np.float32(DK))
NEG = -1.0e30
LSH = 2048.0        # 2^11 lo-part scale
LSHI = float(1.0 / 2048.0)

_CACHE = {}


def _build_program():
    f32 = mybir.dt.float32
    f16 = mybir.dt.float16
    nc = bacc.Bacc("TRN2", target_bir_lowering=False, debug=False, num_devices=8)

    def din(name, shape, dt=f16):
        return nc.dram_tensor(name, shape, dt, kind="ExternalInput").ap()

    xth = din("xth", [DM, S])
    xtl = din("xtl", [DM, S])
    xkh = din("xkh", [DM, NCHUNK * CW])
    xkl = din("xkl", [DM, NCHUNK * CW])
    # wq|wk|wv packed on the free axis to cut DMA-issue count
    wh = din("wh", [DM, 3 * DK])
    wl = din("wl", [DM, 3 * DK])
    dmask = din("dmask", [CW, QSTEP], f32)
    # A~^T chunk-major: rows j*128..(j+1)*128 = chunk j (k within chunk),
    # cols = q. Only q >= 256*j is written; the rest stays zero.
    attn = nc.dram_tensor("attn", [NCHUNK * CW, S], f32, kind="ExternalOutput").ap()
    ot = nc.dram_tensor("ot", [DV, S], f32, kind="ExternalOutput").ap()
    recip_out = nc.dram_tensor("recip", [CW, NCHUNK], f32, kind="ExternalOutput").ap()

    MC = DM // 128  # 8 contraction chunks over d_model

    with tile.TileContext(nc) as tc, ExitStack() as ctx:
        pers = ctx.enter_context(tc.tile_pool(name="pers", bufs=1))
        QT = pers.tile([128, S], f32, name="QT", tag="QT")
        QTh = pers.tile([128, S], f16, name="QTh", tag="QTh")
        QTl = pers.tile([128, S], f16, name="QTl", tag="QTl")
        KTh = pers.tile([128, NCHUNK * CW], f16, name="KTh", tag="KTh")
        KTl = pers.tile([128, NCHUNK * CW], f16, name="KTl", tag="KTl")
        Vt = [pers.tile([128, DV], f32, name=f"V{j}", tag=f"V{j}")
              for j in range(NCHUNK)]
        AT = [pers.tile([128, S - QSTEP * j], f32, name=f"AT{j}", tag=f"AT{j}")
              for j in range(NCHUNK)]
        RECIP = pers.tile([128, NCHUNK], f32, name="RECIP", tag="RECIP")
        DEN = pers.tile([128, NCHUNK], f32, name="DEN", tag="DEN")
        dm = pers.tile([CW, QSTEP], f32, name="dmask", tag="dmask")
        ident = pers.tile([128, 128], f32, name="ident", tag="ident")

        # HAM warmup: ~40 back-to-back tiny matmuls get the PE clock to
        # 2.4GHz (~4us of sustained busy) while input DMAs stream in.
        with tc.tile_pool(name="psW", bufs=1, space="PSUM") as psW:
            make_identity(nc, ident[:])
            wps = psW.tile([128, 128], f32, name="warm", tag="warm")
            for r in range(40):
                nc.tensor.matmul(wps[:], ident[:], ident[:],
                                 start=True, stop=True)

        # ---------------- phase B: projections (split-fp16) ----------------
        with tc.tile_pool(name="pB", bufs=1) as pB, \
             tc.tile_pool(name="psB", bufs=2, space="PSUM") as psB:
            # DMA-issue cost (~0.6us/instruction on one queue engine) gated
            # v3, so: coarser tiles + issue spread over idle engine queues.
            WH, WL = [], []
            for m in range(MC):
                t = pB.tile([128, 3 * DK], f16, name=f"wh{m}", tag=f"wh{m}")
                nc.scalar.dma_start(t[:], wh[m * 128:(m + 1) * 128, :])
                WH.append(t)
            for m in range(MC):
                t = pB.tile([128, 3 * DK], f16, name=f"wl{m}", tag=f"wl{m}")
                nc.scalar.dma_start(t[:], wl[m * 128:(m + 1) * 128, :])
                WL.append(t)
            nc.scalar.dma_start(dm[:], dmask[:, :])
            WQh = [t[:, 0 * DK:1 * DK] for t in WH]
            WKh = [t[:, 1 * DK:2 * DK] for t in WH]
            WVh = [t[:, 2 * DK:3 * DK] for t in WH]
            WQl = [t[:, 0 * DK:1 * DK] for t in WL]
            WKl = [t[:, 1 * DK:2 * DK] for t in WL]
            WVl = [t[:, 2 * DK:3 * DK] for t in WL]

            XTH, XTL = {}, {}
            for hseg in range(2):
                for nm, src, dst in (("xth", xth, XTH), ("xtl", xtl, XTL)):
                    for m in range(MC):
                        t = pB.tile([128, 1024], f16, name=f"{nm}{m}_{hseg}",
                                    tag=f"{nm}{m}_{hseg}")
                        nc.gpsimd.dma_start(
                            t[:], src[m * 128:(m + 1) * 128,
                                      hseg * 1024:(hseg + 1) * 1024])
                        for s in (2 * hseg, 2 * hseg + 1):
                            dst[(m, s)] = t[:, (s % 2) * 512:(s % 2) * 512 + 512]
            XKH, XKL = {}, {}
            for nm, src, dst in (("xkh", xkh, XKH), ("xkl", xkl, XKL)):
                for m in range(MC):
                    t = pB.tile([128, 1024], f16, name=f"{nm}{m}", tag=f"{nm}{m}")
                    nc.sync.dma_start(t[:], src[m * 128:(m + 1) * 128, :])
                    for s in range(2):
                        dst[(m, s)] = t[:, s * 512:s * 512 + 512]

            def proj(dst_ap, seg, WH, WL, XH, XL, s):
                """dst_ap[:, :] = (WH,WL).T @ (XH + 2^-11 XL) for segment s."""
                psm = psB.tile([128, 512], f32, name="psm", tag="psm")
                psl = psB.tile([128, 512], f32, name="psl", tag="psl")
                for m in range(MC):
                    nc.tensor.matmul(psm[:], WH[m][:], XH[(m, s)][:],
                                     start=(m == 0), stop=(m == MC - 1))
                for m in range(MC):
                    nc.tensor.matmul(psl[:], WL[m][:], XH[(m, s)][:],
                                     start=(m == 0), stop=False,
                                     skip_group_check=True)
                for m in range(MC):
                    nc.tensor.matmul(psl[:], WH[m][:], XL[(m, s)][:],
                                     start=False, stop=(m == MC - 1),
                                     skip_group_check=True)
                # combine: dst = psm + 2^-11 * psl (DVE may read only one
                # PSUM operand per op, so evacuate psm first via ACT)
                nc.scalar.copy(dst_ap, psm[:])
                nc.vector.scalar_tensor_tensor(
                    dst_ap, psl[:], LSHI, dst_ap,
                    op0=mybir.AluOpType.mult, op1=mybir.AluOpType.add)

            # Q/K values are O(32): their fp16 lo-parts (~32*2^-11) are well
            # inside fp16 normal range, so no scaling needed on-device.
            for s in range(4):
                proj(QT[:, s * 512:(s + 1) * 512], s, WQh, WQl, XTH, XTL, s)
            for s in range(4):
                sl = slice(s * 512, (s + 1) * 512)
                nc.scalar.copy(QTh[:, sl], QT[:, sl])
                nc.vector.scalar_tensor_tensor(
                    QTl[:, sl], QTh[:, sl], -1.0, QT[:, sl],
                    op0=mybir.AluOpType.mult, op1=mybir.AluOpType.add)

            KT = pers.tile([128, NCHUNK * CW], f32, name="KT", tag="KT")
            for s in range(2):
                proj(KT[:, s * 512:(s + 1) * 512], s, WKh, WKl, XKH, XKL, s)
            for s in range(2):
                sl = slice(s * 512, (s + 1) * 512)
                nc.scalar.copy(KTh[:, sl], KT[:, sl])
                nc.vector.scalar_tensor_tensor(
                    KTl[:, sl], KTh[:, sl], -1.0, KT[:, sl],
                    op0=mybir.AluOpType.mult, op1=mybir.AluOpType.add)

            # V^T then transpose per chunk to V[kl, v]
            VTT = pers.tile([128, NCHUNK * CW], f32, name="VTT", tag="VTT")
            for s in range(2):
                proj(VTT[:, s * 512:(s + 1) * 512], s, WVh, WVl, XKH, XKL, s)
            for j in range(NCHUNK):
                ps = psB.tile([128, 512], f32, name="psm", tag="psm")
                nc.tensor.transpose(ps[:, 0:128],
                                    VTT[:, j * CW:(j + 1) * CW], ident[:])
                nc.vector.tensor_copy(Vt[j][:], ps[:, 0:128])

        # ---------------- phases C (scores/softmax) + D (out) ----------------
        with tc.tile_pool(name="pC", bufs=2) as pC, \
             tc.tile_pool(name="psS", bufs=2, space="PSUM") as psS, \
             tc.tile_pool(name="psO", bufs=1, space="PSUM") as psO:

            pso = psO.tile([128, S], f32, name="O", tag="O")
            OT = pers.tile([128, S], f32, name="OT", tag="OT")

            for j in range(NCHUNK):
                qs0 = QSTEP * j
                lhsh = KTh[:, j * CW:(j + 1) * CW]
                lhsl = KTl[:, j * CW:(j + 1) * CW]
                maxs = pC.tile([128, 8], f32, name="maxs", tag="maxs")
                nseg = 0
                for qs in range(qs0, S, 512):
                    w = min(512, S - qs)
                    # all three fp16 passes accumulate into one PSUM group
                    # (the Q/K lo-parts are unscaled)
                    psm = psS.tile([128, 512], f32, name="Sm", tag="Sm")
                    nc.tensor.matmul(psm[:, 0:w], lhsh, QTh[:, qs:qs + w],
                                     start=True, stop=False)
                    nc.tensor.matmul(psm[:, 0:w], lhsl, QTh[:, qs:qs + w],
                                     start=False, stop=False)
                    nc.tensor.matmul(psm[:, 0:w], lhsh, QTl[:, qs:qs + w],
                                     start=False, stop=True)
                    dst = AT[j][:, qs - qs0:qs - qs0 + w]
                    nc.scalar.copy(dst, psm[:, 0:w])
                    if qs == qs0:
                        nc.vector.tensor_add(AT[j][:, 0:QSTEP],
                                             AT[j][:, 0:QSTEP], dm[:])
                    nc.vector.reduce_max(maxs[:, nseg:nseg + 1],
                                         AT[j][:, qs - qs0:qs - qs0 + w],
                                         axis=mybir.AxisListType.X)
                    nseg += 1
                m = pC.tile([128, 1], f32, name="m", tag="m")
                nc.vector.reduce_max(m[:], maxs[:, 0:nseg],
                                     axis=mybir.AxisListType.X)
                bias = pC.tile([128, 1], f32, name="bias", tag="bias")
                nc.scalar.mul(bias[:], m[:], -INV_SQRT_DK)
                nc.scalar.activation(AT[j][:], AT[j][:],
                                     mybir.ActivationFunctionType.Exp,
                                     bias=bias[:], scale=float(INV_SQRT_DK),
                                     accum_out=DEN[:, j:j + 1])
                nc.vector.reciprocal(RECIP[:, j:j + 1], DEN[:, j:j + 1])

                # store A~^T chunk rows directly (host transposes)
                nc.sync.dma_start(attn[j * CW:(j + 1) * CW, qs0:S], AT[j][:])

                # inline O^T accumulation for this chunk (fp32)
                vs = pers.tile([128, DV], f32, name=f"VS{j}", tag=f"VS{j}")
                nc.vector.tensor_scalar_mul(vs[:], Vt[j][:], RECIP[:, j:j + 1])
                pieces = []
                qs = qs0
                if qs % 512:
                    pieces.append((qs, 512 - qs % 512))
                    qs += 512 - qs % 512
                while qs < S:
                    pieces.append((qs, min(512, S - qs)))
                    qs += 512
                for qs, w in pieces:
                    nc.tensor.matmul(pso[:, qs:qs + w], vs[:],
                                     AT[j][:, qs - qs0:qs - qs0 + w],
                                     start=(j == 0), stop=(j == NCHUNK - 1),
                                     skip_group_check=True)
                # O^T 512-seg t is complete once chunk 2t+1 contributed:
                # evacuate + store early so the tail only carries the last seg.
                if j % 2 == 1:
                    t = j // 2
                    sl = slice(t * 512, (t + 1) * 512)
                    nc.scalar.copy(OT[:, sl], pso[:, sl])
                    nc.sync.dma_start(ot[:, sl], OT[:, sl])

            nc.sync.dma_start(recip_out[:, :], RECIP[:])

    nc.compile()
    return nc


def _get_program():
    if "nc" not in _CACHE:
        _CACHE["nc"] = _build_program()
    return _CACHE["nc"]


def _split16(a):
    hi = a.astype(np.float16)
    lo = ((a - hi.astype(np.float32)) * np.float32(LSH)).astype(np.float16)
    return hi, lo


def _core_inputs(x_q, splits, c):
    b, h = divmod(c, 2)
    xth, xtl = splits[f"xt{b}"]
    cols = np.concatenate(
        [np.arange(QSTEP * j + CW * h, QSTEP * j + CW * h + CW)
         for j in range(NCHUNK)])
    p = np.arange(CW)
    ql = np.arange(QSTEP)
    dmask = np.where(ql[None, :] >= (CW * h + p)[:, None],
                     np.float32(0.0), np.float32(NEG)).astype(np.float32)
    d = {"xth": xth, "xtl": xtl,
         "xkh": np.ascontiguousarray(xth[:, cols]),
         "xkl": np.ascontiguousarray(xtl[:, cols]),
         "wh": splits["wh"], "wl": splits["wl"],
         "dmask": dmask}
    return d


def kernel(x_q, w_q, w_k, w_v, _trace=False, _results_hook=None):
    x_q = np.ascontiguousarray(x_q, dtype=np.float32)

    splits = {}
    for b in range(B):
        splits[f"xt{b}"] = _split16(np.ascontiguousarray(x_q[b].T))
    wpack = np.concatenate([np.asarray(w, dtype=np.float32)
                            for w in (w_q, w_k, w_v)], axis=1)
    splits["wh"], splits["wl"] = _split16(np.ascontiguousarray(wpack))

    nc = _get_program()
    in_maps = [_core_inputs(x_q, splits, c) for c in range(8)]
    res = run_bass_kernel_spmd(nc, in_maps, list(range(8)), trace=_trace)
    if _results_hook is not None:
        _results_hook(res)

    attention = np.zeros((B, S, S), dtype=np.float32)
    out = np.empty((B, S, DV), dtype=np.float32)
    for b in range(B):
        o = None
        for h in range(2):
            c = 2 * b + h
            r = res.results[c]
            a_raw = r["attn"]                 # [1024, S] chunk-major, [k, q]
            rc = r["recip"]                   # [128, 8]
            for j in range(NCHUNK):
                k0 = QSTEP * j + CW * h
                qs0 = QSTEP * j
                blk = a_raw[j * CW:(j + 1) * CW, qs0:] * rc[:, j][:, None]
                attention[b][qs0:, k0:k0 + CW] = blk.T
            oc = r["ot"]                      # [DV, S]
            o = oc if o is None else o + oc
        out[b] = o.T
    return out, attention
